# revision 1
# baseline (speedup 1.0000x reference)
"""Trainium2 Bass kernel for nn_BaseImplicitConv.

out = fft_conv(u, filt) * (u @ pw^T + pb) + u,  filt = MLP(pos_emb)

Sharding: 8 cores = 4 batches x 2 d-halves. Each core computes the
d_model x d_model projection for its (batch, 512-column half) on the
tensor engine (contraction over all 1024 d, PSUM-accumulated), then the
elementwise gate + residual on the vector engine.
"""

import math
import os
import sys

import numpy as np

sys.path.insert(0, "/opt/trn_rl_repo")
sys.path.insert(0, "/opt/trn_rl_repo/concourse")

import concourse.bass as bass
import concourse.mybir as mybir
from concourse.bass_utils import run_bass_kernel_spmd
from concourse import tile
from concourse.vector_clock import ScopedClock
import bass_rust

B, L, D = 4, 4096, 1024
N_CORES = 8
HALF = D // 2  # 512 columns per core


def _patch_tile_drain():
    """walrus in this container rejects >1 sync-wait on a CTRL (Drain)
    instruction; emit each wait on its own NOP instead."""

    def _drain_and_barrier(self, tick_clock, wait_clock):
        drain_inst = self.nc.sync.drain()
        wait_clock.add_sem_waits(
            drain_inst.ins, ScopedClock({None: tick_clock.global_clock})
        )
        si = drain_inst.ins.sync_info
        if si is not None and len(si.on_wait) > 1:
            waits = list(si.on_wait)
            drain_inst.ins.sync_info = bass_rust.SyncInfo(
                on_wait=[], on_update=list(si.on_update)
            )
            for w in waits:
                wi = self.nc.sync.nop(nofuse=True)
                wi.ins.sync_info = bass_rust.SyncInfo(on_wait=[w], on_update=[])
        self.nc.all_engine_barrier()
        assert self.sems is not None
        popped = self.nc._tile_sem_poison_stack.pop()
        assert popped is self._sem_poison
        self.nc.clear_and_free_semaphores(list(self.sems.allocated().values()))
        self.nc.all_engine_barrier()

    tile.TileContext._drain_and_barrier = _drain_and_barrier


_patch_tile_drain()

_SPLIT_CTR = [0]


def _split_multi_waits(nc):
    """This walrus build allows at most one sync-wait per instruction; hoist
    extras onto same-engine NOPs placed immediately before the instruction."""
    for f in nc.m.functions:
        for bb in f.blocks:
            new_insts = []
            changed = False
            for inst in bb.instructions:
                si = inst.sync_info
                if si is not None and len(si.on_wait) > 1:
                    waits = list(si.on_wait)
                    for w in waits[:-1]:
                        _SPLIT_CTR[0] += 1
                        nop = mybir.InstNoOp(
                            name=f"wsplit-{_SPLIT_CTR[0]}", ins=[], outs=[]
                        )
                        nop.engine = inst.engine
                        nop.sync_info = bass_rust.SyncInfo(
                            on_wait=[w], on_update=[]
                        )
                        nc.register_instruction(nop, overwrite=True)
                        new_insts.append(nop)
                    inst.sync_info = bass_rust.SyncInfo(
                        on_wait=[waits[-1]], on_update=list(si.on_update)
                    )
                    changed = True
                new_insts.append(inst)
            if changed:
                bb.instructions = new_insts


_NC_CACHE = {}


def _build_nc():
    """Per-core Bass module: out[l,o] = yconv[l,o] * (sum_d uT[d,l]*pwT[d,o]) + rest[l,o]."""
    if "nc" in _NC_CACHE:
        return _NC_CACHE["nc"]
    nc = bass.Bass()
    f32 = mybir.dt.float32
    uT = nc.dram_tensor("uT", [D, L], f32, kind="ExternalInput")
    pwT = nc.dram_tensor("pwT", [D, HALF], f32, kind="ExternalInput")
    yconv = nc.dram_tensor("yconv", [L, HALF], f32, kind="ExternalInput")
    rest = nc.dram_tensor("rest", [L, HALF], f32, kind="ExternalInput")
    out = nc.dram_tensor("out", [L, HALF], f32, kind="ExternalOutput")

    KT = D // 128  # 8 contraction tiles
    LT = L // 128  # 32 output row tiles

    with tile.TileContext(nc) as tc:
        with (
            tc.tile_pool(name="pw", bufs=1) as pw_pool,
            tc.tile_pool(name="ut", bufs=3) as ut_pool,
            tc.tile_pool(name="io", bufs=4) as io_pool,
            tc.tile_pool(name="ps", bufs=4, space="PSUM") as ps_pool,
        ):
            pw_t = pw_pool.tile([128, KT * HALF], f32)
            # pwT DRAM (D, HALF): partition = d%128, free packs (ktile, o)
            nc.sync.dma_start(
                out=pw_t[:].rearrange("p (k o) -> p k o", k=KT),
                in_=pwT.rearrange("(k p) o -> p k o", p=128)
            )
            for lt in range(LT):
                # uT[:, lt*128:+128] -> [128 part = d%128, (ktile, l)]
                ut_t = ut_pool.tile([128, KT * 128], f32)
                nc.sync.dma_start(
                    out=ut_t[:].rearrange("p (k l) -> p k l", k=KT),
                    in_=uT[:, lt * 128 : (lt + 1) * 128].rearrange(
                        "(k p) l -> p k l", p=128
                    ),
                )
                ps = ps_pool.tile([128, HALF], f32)
                for kt in range(KT):
                    nc.tensor.matmul(
                        ps[:],
                        ut_t[:, kt * 128 : (kt + 1) * 128],
                        pw_t[:, kt * HALF : (kt + 1) * HALF],
                        start=(kt == 0),
                        stop=(kt == KT - 1),
                    )
                y_t = io_pool.tile([128, HALF], f32, tag="y")
                r_t = io_pool.tile([128, HALF], f32, tag="r")
                o_t = io_pool.tile([128, HALF], f32, tag="o")
                nc.sync.dma_start(
                    out=y_t[:], in_=yconv[lt * 128 : (lt + 1) * 128, :]
                )
                nc.sync.dma_start(
                    out=r_t[:], in_=rest[lt * 128 : (lt + 1) * 128, :]
                )
                nc.vector.tensor_mul(o_t[:], y_t[:], ps[:])
                nc.vector.tensor_add(o_t[:], o_t[:], r_t[:])
                nc.sync.dma_start(
                    out=out[lt * 128 : (lt + 1) * 128, :], in_=o_t[:]
                )
    _split_multi_waits(nc)
    _NC_CACHE["nc"] = nc
    return nc


def kernel(**inputs):
    u = np.asarray(inputs["u"], dtype=np.float32)
    z = np.asarray(inputs["z"], dtype=np.float32)
    w1 = np.asarray(inputs["w1"], dtype=np.float32)
    b1 = np.asarray(inputs["b1"], dtype=np.float32)
    w2 = np.asarray(inputs["w2"], dtype=np.float32)
    b2 = np.asarray(inputs["b2"], dtype=np.float32)
    pw = np.asarray(inputs["pw"], dtype=np.float32)
    pb = np.asarray(inputs["pb"], dtype=np.float32)

    # filter MLP (tiny) + FFT conv spectra prep on host
    pe = z[:, :L]  # (1, L, 3)
    h = np.maximum(pe @ w1.T + b1, 0.0)  # (1, L, 16)
    filt = (h @ w2.T + b2)[0]  # (L, D)
    k_f = np.fft.rfft(filt.T, n=2 * L)  # (D, 4097)
    u_t = np.transpose(u, (0, 2, 1))  # (B, D, L)
    u_f = np.fft.rfft(u_t, n=2 * L)
    y = np.fft.irfft(u_f * k_f, n=2 * L)[..., :L]  # (B, D, L) causal conv
    y_ld = np.ascontiguousarray(
        np.transpose(y, (0, 2, 1)), dtype=np.float32
    )  # (B, L, D)

    pwT = np.ascontiguousarray(pw.T)  # (D, D): pwT[d, o] = pw[o, d]

    in_maps = []
    for c in range(N_CORES):
        b, hf = c // 2, c % 2
        sl = slice(hf * HALF, (hf + 1) * HALF)
        yc = y_ld[b][:, sl]
        in_maps.append(
            {
                "uT": np.ascontiguousarray(u[b].T),
                "pwT": np.ascontiguousarray(pwT[:, sl]),
                "yconv": np.ascontiguousarray(yc),
                "rest": np.ascontiguousarray(yc * pb[sl] + u[b][:, sl]),
            }
        )

    nc = _build_nc()
    res = run_bass_kernel_spmd(nc, in_maps, list(range(N_CORES)))

    out = np.empty((B, L, D), dtype=np.float32)
    for c in range(N_CORES):
        b, hf = c // 2, c % 2
        out[b, :, hf * HALF : (hf + 1) * HALF] = res.results[c]["out"]
    return out



# revision 2
# speedup vs baseline: 1.8980x; 1.8980x over previous
"""Trainium2 Bass kernel for nn_BaseImplicitConv (wire-optimized).

out = fft_conv(u, filt) * (u @ pw^T + pb) + u,  filt = MLP(pos_emb)

The axon tunnel moves ~35-40 MB/s, so wall time is dominated by wire
bytes.  v2 ships everything in fp16 (rel-err budget 2e-2 vs fp16's
~5e-4), shards (batch, L-half) so u ships exactly once (no duplicate
shards), and keeps host work in the native (l, d) layout (scipy f32
FFT along axis=1, contiguous per-core slices, no host transposes).

Per core: rows l of one batch.  Device transposes u tiles with the
tensor engine (is_transpose matmul vs identity) to get the d-major
stationary tiles the projection matmul needs; pb enters as the K=1
PSUM-init matmul ones^T @ pb; gate+residual run on the vector engine
in the native (l-part, d-free) orientation so the output DMA is
contiguous.
"""

import math
import sys

import numpy as np

sys.path.insert(0, "/opt/trn_rl_repo")
sys.path.insert(0, "/opt/trn_rl_repo/concourse")

import concourse.bass as bass
import concourse.mybir as mybir
from concourse.bass_utils import run_bass_kernel_spmd
from concourse import tile
from concourse.vector_clock import ScopedClock
import bass_rust

B, L, D = 4, 4096, 1024
N_CORES = 8
LC = L // 2  # 2048 rows per core
NLC = LC // 128  # 16 l-chunks
KT = D // 128  # 8 contraction tiles
OHALF = 512  # PSUM free width (one bank of fp32)


def _patch_tile_drain():
    """walrus in this container rejects >1 sync-wait on a CTRL (Drain)
    instruction; emit each wait on its own NOP instead."""

    def _drain_and_barrier(self, tick_clock, wait_clock):
        drain_inst = self.nc.sync.drain()
        wait_clock.add_sem_waits(
            drain_inst.ins, ScopedClock({None: tick_clock.global_clock})
        )
        si = drain_inst.ins.sync_info
        if si is not None and len(si.on_wait) > 1:
            waits = list(si.on_wait)
            drain_inst.ins.sync_info = bass_rust.SyncInfo(
                on_wait=[], on_update=list(si.on_update)
            )
            for w in waits:
                wi = self.nc.sync.nop(nofuse=True)
                wi.ins.sync_info = bass_rust.SyncInfo(on_wait=[w], on_update=[])
        self.nc.all_engine_barrier()
        assert self.sems is not None
        popped = self.nc._tile_sem_poison_stack.pop()
        assert popped is self._sem_poison
        self.nc.clear_and_free_semaphores(list(self.sems.allocated().values()))
        self.nc.all_engine_barrier()

    tile.TileContext._drain_and_barrier = _drain_and_barrier


_patch_tile_drain()

_SPLIT_CTR = [0]


def _split_multi_waits(nc):
    """This walrus build allows at most one sync-wait per instruction; hoist
    extras onto same-engine NOPs placed immediately before the instruction."""
    for f in nc.m.functions:
        for bb in f.blocks:
            new_insts = []
            changed = False
            for inst in bb.instructions:
                si = inst.sync_info
                if si is not None and len(si.on_wait) > 1:
                    waits = list(si.on_wait)
                    for w in waits[:-1]:
                        _SPLIT_CTR[0] += 1
                        nop = mybir.InstNoOp(
                            name=f"wsplit-{_SPLIT_CTR[0]}", ins=[], outs=[]
                        )
                        nop.engine = inst.engine
                        nop.sync_info = bass_rust.SyncInfo(
                            on_wait=[w], on_update=[]
                        )
                        nc.register_instruction(nop, overwrite=True)
                        new_insts.append(nop)
                    inst.sync_info = bass_rust.SyncInfo(
                        on_wait=[waits[-1]], on_update=list(si.on_update)
                    )
                    changed = True
                new_insts.append(inst)
            if changed:
                bb.instructions = new_insts


_NC_CACHE = {}


def _build_nc():
    """Per-core module: out16 = yc16 * (u16 @ pwT16 + pb16) + u16, fp16 I/O."""
    if "nc" in _NC_CACHE:
        return _NC_CACHE["nc"]
    nc = bass.Bass()
    f16 = mybir.dt.float16
    f32 = mybir.dt.float32
    u16 = nc.dram_tensor("u16", [LC, D], f16, kind="ExternalInput")
    yc16 = nc.dram_tensor("yc16", [LC, D], f16, kind="ExternalInput")
    pwT16 = nc.dram_tensor("pwT16", [D, D], f16, kind="ExternalInput")
    pb16 = nc.dram_tensor("pb16", [1, D], f16, kind="ExternalInput")
    ident = nc.dram_tensor("ident", [128, 128], f16, kind="ExternalInput")
    out16 = nc.dram_tensor("out16", [LC, D], f16, kind="ExternalOutput")

    with tile.TileContext(nc) as tc:
        with (
            tc.tile_pool(name="const", bufs=1) as const_pool,
            tc.tile_pool(name="u", bufs=3) as u_pool,
            tc.tile_pool(name="yc", bufs=3) as yc_pool,
            tc.tile_pool(name="ut", bufs=3) as ut_pool,
            tc.tile_pool(name="g", bufs=4) as g_pool,
            tc.tile_pool(name="pst", bufs=4, space="PSUM") as pst_pool,
            tc.tile_pool(name="ps", bufs=4, space="PSUM") as ps_pool,
        ):
            # pwT DRAM (D, D): partition = d%128, free packs (ktile, o)
            pw_t = const_pool.tile([128, KT * D], f16)
            nc.sync.dma_start(
                out=pw_t[:].rearrange("p (k o) -> p k o", k=KT),
                in_=pwT16.rearrange("(k p) o -> p k o", p=128),
            )
            pb_t = const_pool.tile([1, D], f16)
            nc.sync.dma_start(out=pb_t[:], in_=pb16[:, :])
            id_t = const_pool.tile([128, 128], f16)
            nc.sync.dma_start(out=id_t[:], in_=ident[:, :])
            ones_t = const_pool.tile([1, 128], f16)
            nc.vector.memset(ones_t[:], 1.0)

            for lc in range(NLC):
                rows = slice(lc * 128, (lc + 1) * 128)
                u_t = u_pool.tile([128, D], f16)
                nc.sync.dma_start(out=u_t[:], in_=u16[rows, :])
                yc_t = yc_pool.tile([128, D], f16)
                nc.sync.dma_start(out=yc_t[:], in_=yc16[rows, :])

                # transpose u tiles: (l-part, d) -> (d-part, l) for matmul
                uT_t = ut_pool.tile([128, KT * 128], f16)
                for kt in range(KT):
                    ksl = slice(kt * 128, (kt + 1) * 128)
                    pst = pst_pool.tile([128, 128], f16)
                    nc.tensor.transpose(pst[:], u_t[:, ksl], id_t[:])
                    nc.scalar.copy(uT_t[:, ksl], pst[:])

                for oh in range(2):
                    osl = slice(oh * OHALF, (oh + 1) * OHALF)
                    ps = ps_pool.tile([128, OHALF], f32)
                    # PSUM init = broadcast pb row: ones(1,128)^T @ pb(1,512)
                    nc.tensor.matmul(
                        ps[:],
                        ones_t[:1, :],
                        pb_t[:1, osl],
                        start=True,
                        stop=False,
                    )
                    for kt in range(KT):
                        nc.tensor.matmul(
                            ps[:],
                            uT_t[:, kt * 128 : (kt + 1) * 128],
                            pw_t[:, kt * D + oh * OHALF : kt * D + oh * OHALF + OHALF],
                            start=False,
                            stop=(kt == KT - 1),
                        )
                    g = g_pool.tile([128, OHALF], f16)
                    nc.vector.tensor_mul(g[:], yc_t[:, osl], ps[:])
                    nc.vector.tensor_add(g[:], g[:], u_t[:, osl])
                    nc.sync.dma_start(out=out16[rows, osl], in_=g[:])
    _split_multi_waits(nc)
    _NC_CACHE["nc"] = nc
    return nc


def kernel(**inputs):
    import scipy.fft as sfft

    u = np.asarray(inputs["u"], dtype=np.float32)
    z = np.asarray(inputs["z"], dtype=np.float32)
    w1 = np.asarray(inputs["w1"], dtype=np.float32)
    b1 = np.asarray(inputs["b1"], dtype=np.float32)
    w2 = np.asarray(inputs["w2"], dtype=np.float32)
    b2 = np.asarray(inputs["b2"], dtype=np.float32)
    pw = np.asarray(inputs["pw"], dtype=np.float32)
    pb = np.asarray(inputs["pb"], dtype=np.float32)

    # filter MLP (tiny) on host
    pe = z[0, :L]  # (L, 3)
    h = np.maximum(pe @ w1.T + b1, 0.0)  # (L, 16)
    filt = h @ w2.T + b2  # (L, D)

    # causal FFT conv along axis=1 in native (b, l, d) layout, float32
    k_f = sfft.rfft(filt, n=2 * L, axis=0)  # (L+1, D) c64
    u_f = sfft.rfft(u, n=2 * L, axis=1)  # (B, L+1, D) c64
    np.multiply(u_f, k_f[None], out=u_f)
    y = sfft.irfft(u_f, n=2 * L, axis=1)  # (B, 2L, D) f32; rows >= L unused

    pwT16 = pw.T.astype(np.float16)  # (D, D), pwT[d, o] = pw[o, d]
    pb16 = pb.reshape(1, D).astype(np.float16)
    ident = np.eye(128, dtype=np.float16)

    in_maps = []
    for c in range(N_CORES):
        b, hf = c // 2, c % 2
        rows = slice(hf * LC, (hf + 1) * LC)
        in_maps.append(
            {
                "u16": u[b, rows].astype(np.float16),
                "yc16": y[b, rows].astype(np.float16),
                "pwT16": pwT16,
                "pb16": pb16,
                "ident": ident,
            }
        )

    nc = _build_nc()
    res = run_bass_kernel_spmd(nc, in_maps, list(range(N_CORES)))

    out = np.empty((B, L, D), dtype=np.float32)
    for c in range(N_CORES):
        b, hf = c // 2, c % 2
        out[b, hf * LC : (hf + 1) * LC] = res.results[c]["out16"]
    return out


# revision 4
# speedup vs baseline: 2.1224x; 1.1183x over previous
"""Trainium2 Bass kernel for nn_BaseImplicitConv (v3 — on-device conv).

out = fft_conv(u, filt) * (u @ pw^T + pb) + u,  filt = MLP(pos_emb)

Key observation: the positional embedding pe is linear in t to ~1e-7
(its cos/sin bands have arguments <= 2*pi*1e-4), so every filter
channel is piecewise-linear in the tap index s with <= 16 breakpoints
shared across channels.  A causal conv with filter a*s+b on the tap
window [sigma, L) reduces exactly to

    C[l] = (b + a*(sigma-1)) * U1[l-sigma] + a * U2[l-sigma]

with U1 = cumsum(u), U2 = cumsum(U1) along l.  Grouping units by
breakpoint gives y[l,c] = sum_j P_j(c)*U1[l-s_j,c] + Q_j(c)*U2[l-s_j,c]
over a handful of shifts — a few prefix scans plus shifted
scaled-adds on the vector engine.  No FFT anywhere, and the conv input
is the same uT the projection matmul contracts, so only u itself (and
tiny coefficients) crosses the slow axon wire.

Sharding: 8 cores = 4 batches x 2 channel-halves.  Each core receives
the full uT[b] in fp16 (rows permuted so its own 512 channels come
first), computes conv + projection + gate for its half in (c-part, l)
orientation throughout, and writes outT fp16.  The host only
transposes u once and reassembles (as a transposed view).

Falls back to a host-FFT path (v2 scheme) if the piecewise-linear
assumptions ever fail for unexpected weight inputs.
"""

import math
import sys

import numpy as np

sys.path.insert(0, "/opt/trn_rl_repo")
sys.path.insert(0, "/opt/trn_rl_repo/concourse")

import concourse.bass as bass
import concourse.mybir as mybir
from concourse.bass_utils import run_bass_kernel_spmd
from concourse import tile
from concourse.vector_clock import ScopedClock
import bass_rust

B, L, D = 4, 4096, 1024
N_CORES = 8
HALF = D // 2  # 512 channels per core
KT = D // 128  # 8 contraction tiles
CT = HALF // 128  # 4 own-channel tiles
NLCH = L // 512  # 8 l-chunks of 512


def _patch_tile_drain():
    """walrus in this container rejects >1 sync-wait on a CTRL (Drain)
    instruction; emit each wait on its own NOP instead."""

    def _drain_and_barrier(self, tick_clock, wait_clock):
        drain_inst = self.nc.sync.drain()
        wait_clock.add_sem_waits(
            drain_inst.ins, ScopedClock({None: tick_clock.global_clock})
        )
        si = drain_inst.ins.sync_info
        if si is not None and len(si.on_wait) > 1:
            waits = list(si.on_wait)
            drain_inst.ins.sync_info = bass_rust.SyncInfo(
                on_wait=[], on_update=list(si.on_update)
            )
            for w in waits:
                wi = self.nc.sync.nop(nofuse=True)
                wi.ins.sync_info = bass_rust.SyncInfo(on_wait=[w], on_update=[])
        self.nc.all_engine_barrier()
        assert self.sems is not None
        popped = self.nc._tile_sem_poison_stack.pop()
        assert popped is self._sem_poison
        self.nc.clear_and_free_semaphores(list(self.sems.allocated().values()))
        self.nc.all_engine_barrier()

    tile.TileContext._drain_and_barrier = _drain_and_barrier


_patch_tile_drain()

_SPLIT_CTR = [0]


def _split_multi_waits(nc):
    """This walrus build allows at most one sync-wait per instruction; hoist
    extras onto same-engine NOPs placed immediately before the instruction."""
    for f in nc.m.functions:
        for bb in f.blocks:
            new_insts = []
            changed = False
            for inst in bb.instructions:
                si = inst.sync_info
                if si is not None and len(si.on_wait) > 1:
                    waits = list(si.on_wait)
                    for w in waits[:-1]:
                        _SPLIT_CTR[0] += 1
                        nop = mybir.InstNoOp(
                            name=f"wsplit-{_SPLIT_CTR[0]}", ins=[], outs=[]
                        )
                        nop.engine = inst.engine
                        nop.sync_info = bass_rust.SyncInfo(
                            on_wait=[w], on_update=[]
                        )
                        nc.register_instruction(nop, overwrite=True)
                        new_insts.append(nop)
                    inst.sync_info = bass_rust.SyncInfo(
                        on_wait=[waits[-1]], on_update=list(si.on_update)
                    )
                    changed = True
                new_insts.append(inst)
            if changed:
                bb.instructions = new_insts


_NC_CACHE = {}


def _build_nc(shifts):
    """Per-core module keyed by the tuple of conv shifts (compile-time AP
    offsets).  outT = yconv * (uT.T @ pwTh + pb) + uT for the core's 512
    channels; yconv from prefix scans U1/U2 and shifted scaled adds."""
    key = tuple(shifts)
    if key in _NC_CACHE:
        return _NC_CACHE[key]
    ns = len(shifts)
    nc = bass.Bass()
    f16 = mybir.dt.float16
    f32 = mybir.dt.float32
    add = mybir.AluOpType.add
    mult = mybir.AluOpType.mult
    bypass = mybir.AluOpType.bypass

    uT16 = nc.dram_tensor("uT16", [D, L], f16, kind="ExternalInput")
    pwTh16 = nc.dram_tensor("pwTh16", [D, HALF], f16, kind="ExternalInput")
    pbh = nc.dram_tensor("pbh", [HALF, 1], f32, kind="ExternalInput")
    P32 = nc.dram_tensor("P32", [HALF, ns], f32, kind="ExternalInput")
    Q32 = nc.dram_tensor("Q32", [HALF, ns], f32, kind="ExternalInput")
    outT16 = nc.dram_tensor("outT16", [HALF, L], f16, kind="ExternalOutput")

    with tile.TileContext(nc) as tc:
        with (
            tc.tile_pool(name="const", bufs=1) as const_pool,
            tc.tile_pool(name="scan", bufs=2) as scan_pool,
            tc.tile_pool(name="acc", bufs=2) as acc_pool,
            tc.tile_pool(name="g", bufs=4) as g_pool,
            tc.tile_pool(name="ps", bufs=4, space="PSUM") as ps_pool,
        ):
            # uT DRAM (D, L): partition = d%128, free packs (ktile, l)
            u_t = const_pool.tile([128, KT * L], f16)
            nc.sync.dma_start(
                out=u_t[:].rearrange("p (k l) -> p k l", k=KT),
                in_=uT16.rearrange("(k p) l -> p k l", p=128),
            )
            # pwTh DRAM (D, HALF): same kt packing for matmul stationaries
            pw_t = const_pool.tile([128, KT * HALF], f16)
            nc.sync.dma_start(
                out=pw_t[:].rearrange("p (k o) -> p k o", k=KT),
                in_=pwTh16.rearrange("(k p) o -> p k o", p=128),
            )
            pb_t = const_pool.tile([128, CT], f32)
            nc.sync.dma_start(
                out=pb_t[:].rearrange("p (k j) -> p k j", k=CT),
                in_=pbh.rearrange("(k p) j -> p k j", p=128),
            )
            p_t = const_pool.tile([128, CT * ns], f32)
            nc.sync.dma_start(
                out=p_t[:].rearrange("p (k j) -> p k j", k=CT),
                in_=P32.rearrange("(k p) j -> p k j", p=128),
            )
            q_t = const_pool.tile([128, CT * ns], f32)
            nc.sync.dma_start(
                out=q_t[:].rearrange("p (k j) -> p k j", k=CT),
                in_=Q32.rearrange("(k p) j -> p k j", p=128),
            )

            for ct in range(CT):
                u_ct = u_t[:, ct * L : (ct + 1) * L]
                # prefix sums along l (fp32 scan state)
                u1 = scan_pool.tile([128, L], f32, tag="u1")
                nc.vector.tensor_tensor_scan(
                    u1[:], u_ct, u_ct, 0.0, add, bypass
                )
                u2 = scan_pool.tile([128, L], f32, tag="u2")
                nc.vector.tensor_tensor_scan(
                    u2[:], u1[:], u1[:], 0.0, add, bypass
                )
                # y = sum_j P_j*U1[l-s_j] + Q_j*U2[l-s_j]; shifts[0] == 0
                acc = acc_pool.tile([128, L], f32)
                nc.vector.tensor_scalar_mul(
                    acc[:], u1[:], p_t[:, ct * ns : ct * ns + 1]
                )
                nc.vector.scalar_tensor_tensor(
                    acc[:], u2[:], q_t[:, ct * ns : ct * ns + 1], acc[:],
                    mult, add,
                )
                for j in range(1, ns):
                    sg = shifts[j]
                    w = L - sg
                    nc.vector.scalar_tensor_tensor(
                        acc[:, sg:], u1[:, :w],
                        p_t[:, ct * ns + j : ct * ns + j + 1],
                        acc[:, sg:], mult, add,
                    )
                    nc.vector.scalar_tensor_tensor(
                        acc[:, sg:], u2[:, :w],
                        q_t[:, ct * ns + j : ct * ns + j + 1],
                        acc[:, sg:], mult, add,
                    )
                # projection + gate per 512-wide l-chunk
                for lc in range(NLCH):
                    lsl = slice(lc * 512, (lc + 1) * 512)
                    ps = ps_pool.tile([128, 512], f32)
                    for kt in range(KT):
                        nc.tensor.matmul(
                            ps[:],
                            pw_t[:, kt * HALF + ct * 128 : kt * HALF + (ct + 1) * 128],
                            u_t[:, kt * L + lc * 512 : kt * L + lc * 512 + 512],
                            start=(kt == 0),
                            stop=(kt == KT - 1),
                        )
                    g = g_pool.tile([128, 512], f16)
                    # (proj + pb) * yconv
                    nc.vector.scalar_tensor_tensor(
                        g[:], ps[:], pb_t[:, ct : ct + 1], acc[:, lsl],
                        add, mult,
                    )
                    nc.vector.tensor_add(g[:], g[:], u_ct[:, lsl])
                    nc.sync.dma_start(
                        out=outT16[ct * 128 : (ct + 1) * 128, lsl], in_=g[:]
                    )
    _split_multi_waits(nc)
    _NC_CACHE[key] = nc
    return nc


def _conv_coeffs(z, w1, b1, w2, b2):
    """Piecewise-linear decomposition of the implicit filter.

    Returns (shifts, P, Q) with shifts[0] == 0 and P/Q of shape
    (len(shifts), D), such that
    y[l, c] = sum_j P[j, c]*U1[l-s_j, c] + Q[j, c]*U2[l-s_j, c].
    Returns None if the filter is not piecewise-linear in this form.
    """
    pe = z[0, :L].astype(np.float64)  # (L, 3)
    g = pe @ w1.T.astype(np.float64) + b1.astype(np.float64)  # (L, H)
    s_idx = np.arange(L, dtype=np.float64)
    A = np.stack([s_idx, np.ones(L)], axis=1)
    coef, *_ = np.linalg.lstsq(A, g, rcond=None)
    if np.abs(g - A @ coef).max() > 1e-5:
        return None
    a_u, b_u = coef[0], coef[1]
    P = {0: b2.astype(np.float64).copy()}
    Q = {0: np.zeros(D, np.float64)}
    active = g > 0
    for hh in range(g.shape[1]):
        al, be = a_u[hh], b_u[hh]
        act = active[:, hh]
        if not act.any():
            continue
        w2h = w2[:, hh].astype(np.float64)
        if act.all():
            P[0] += w2h * (be - al)
            Q[0] += w2h * al
            continue
        if np.count_nonzero(act[1:] != act[:-1]) != 1:
            return None
        if act[-1] and not act[0]:  # active suffix [sig, L)
            sig = int(np.argmax(act))
            P.setdefault(sig, np.zeros(D, np.float64))
            Q.setdefault(sig, np.zeros(D, np.float64))
            P[sig] += w2h * (be + al * (sig - 1))
            Q[sig] += w2h * al
        else:  # active prefix [0, sig)
            sig = int(np.argmax(~act))
            P[0] += w2h * (be - al)
            Q[0] += w2h * al
            P.setdefault(sig, np.zeros(D, np.float64))
            Q.setdefault(sig, np.zeros(D, np.float64))
            P[sig] -= w2h * (be + al * (sig - 1))
            Q[sig] -= w2h * al
    shifts = sorted(P.keys())
    Pm = np.stack([P[s] for s in shifts]).astype(np.float32)
    Qm = np.stack([Q[s] for s in shifts]).astype(np.float32)
    return shifts, Pm, Qm


def kernel(**inputs):
    u = np.asarray(inputs["u"], dtype=np.float32)
    z = np.asarray(inputs["z"], dtype=np.float32)
    w1 = np.asarray(inputs["w1"], dtype=np.float32)
    b1 = np.asarray(inputs["b1"], dtype=np.float32)
    w2 = np.asarray(inputs["w2"], dtype=np.float32)
    b2 = np.asarray(inputs["b2"], dtype=np.float32)
    pw = np.asarray(inputs["pw"], dtype=np.float32)
    pb = np.asarray(inputs["pb"], dtype=np.float32)

    cc = _conv_coeffs(z, w1, b1, w2, b2)
    if cc is None:  # unexpected weights: exact host fallback
        pe = z[:, :L]
        h = np.maximum(np.einsum("ble,he->blh", pe, w1) + b1, 0.0)
        filt = (np.einsum("blh,dh->bld", h, w2) + b2)[0].T  # (D, L)
        k_f = np.fft.rfft(filt, n=2 * L)
        u_t = u.transpose(0, 2, 1)
        y = np.fft.irfft(np.fft.rfft(u_t, n=2 * L) * k_f, n=2 * L)[..., :L]
        proj = np.einsum("bld,od->blo", u, pw) + pb
        return (y.transpose(0, 2, 1) * proj + u).astype(np.float32)
    shifts, Pm, Qm = cc
    ns = len(shifts)

    pwT16 = pw.T.astype(np.float16)  # (D, D), pwT[d, o] = pw[o, d]
    ut = np.ascontiguousarray(u.transpose(0, 2, 1))  # (B, D, L)
    ut16 = [ut[b].astype(np.float16) for b in range(B)]

    in_maps = []
    for c in range(N_CORES):
        b, hf = c // 2, c % 2
        own = slice(hf * HALF, (hf + 1) * HALF)
        other = slice((1 - hf) * HALF, (2 - hf) * HALF)
        in_maps.append(
            {
                # own channels first so the SPMD program is uniform
                "uT16": np.concatenate([ut16[b][own], ut16[b][other]]),
                "pwTh16": np.concatenate([pwT16[own], pwT16[other]])[:, own],
                "pbh": pb[own].reshape(HALF, 1).astype(np.float32),
                "P32": np.ascontiguousarray(Pm[:, own].T),
                "Q32": np.ascontiguousarray(Qm[:, own].T),
            }
        )

    nc = _build_nc(shifts)
    res = run_bass_kernel_spmd(nc, in_maps, list(range(N_CORES)))

    outT = np.empty((B, D, L), dtype=np.float32)
    for c in range(N_CORES):
        b, hf = c // 2, c % 2
        outT[b, hf * HALF : (hf + 1) * HALF] = res.results[c]["outT16"]
    return outT.transpose(0, 2, 1)


# revision 5
# speedup vs baseline: 3.2380x; 1.5256x over previous
"""Trainium2 Bass kernel for nn_BaseImplicitConv (v4 — ReduceScatter proj).

Same scheme as v3 (piecewise-linear conv via prefix scans; see
kernel3.py), but u ships over the slow axon wire exactly once: each
core receives only its own 512-channel half of uT[b].  The d x d
projection contracts over all 1024 channels, so each core computes the
partial projection over its half for ALL output columns and a pairwise
fp16 ReduceScatter(add) over {2b, 2b+1} yields the full projection
rows each core gates with.  Channel halves follow global order, so the
SPMD program is identical on every core (even cores reduce-scatter
into rank 0 = columns [0, 512), odd into [512, 1024)).
"""

import math
import sys

import numpy as np

sys.path.insert(0, "/opt/trn_rl_repo")
sys.path.insert(0, "/opt/trn_rl_repo/concourse")

import concourse.bass as bass
import concourse.mybir as mybir
from concourse.bass_utils import run_bass_kernel_spmd
from concourse import tile
from concourse.vector_clock import ScopedClock
import bass_rust

B, L, D = 4, 4096, 1024
N_CORES = 8
HALF = D // 2  # 512 channels per core
KTH = HALF // 128  # 4 own-channel contraction tiles
CT = KTH
NLCH = L // 512  # 8 l-chunks of 512
NOC = D // 128  # 8 output-column chunks of the partial projection


def _patch_tile_drain():
    """walrus in this container rejects >1 sync-wait on a CTRL (Drain)
    instruction; emit each wait on its own NOP instead."""

    def _drain_and_barrier(self, tick_clock, wait_clock):
        drain_inst = self.nc.sync.drain()
        wait_clock.add_sem_waits(
            drain_inst.ins, ScopedClock({None: tick_clock.global_clock})
        )
        si = drain_inst.ins.sync_info
        if si is not None and len(si.on_wait) > 1:
            waits = list(si.on_wait)
            drain_inst.ins.sync_info = bass_rust.SyncInfo(
                on_wait=[], on_update=list(si.on_update)
            )
            for w in waits:
                wi = self.nc.sync.nop(nofuse=True)
                wi.ins.sync_info = bass_rust.SyncInfo(on_wait=[w], on_update=[])
        self.nc.all_engine_barrier()
        assert self.sems is not None
        popped = self.nc._tile_sem_poison_stack.pop()
        assert popped is self._sem_poison
        self.nc.clear_and_free_semaphores(list(self.sems.allocated().values()))
        self.nc.all_engine_barrier()

    tile.TileContext._drain_and_barrier = _drain_and_barrier


_patch_tile_drain()

_SPLIT_CTR = [0]


def _split_multi_waits(nc):
    """This walrus build allows at most one sync-wait per instruction; hoist
    extras onto same-engine NOPs placed immediately before the instruction."""
    for f in nc.m.functions:
        for bb in f.blocks:
            new_insts = []
            changed = False
            for inst in bb.instructions:
                si = inst.sync_info
                if si is not None and len(si.on_wait) > 1:
                    waits = list(si.on_wait)
                    for w in waits[:-1]:
                        _SPLIT_CTR[0] += 1
                        nop = mybir.InstNoOp(
                            name=f"wsplit-{_SPLIT_CTR[0]}", ins=[], outs=[]
                        )
                        nop.engine = inst.engine
                        nop.sync_info = bass_rust.SyncInfo(
                            on_wait=[w], on_update=[]
                        )
                        nc.register_instruction(nop, overwrite=True)
                        new_insts.append(nop)
                    inst.sync_info = bass_rust.SyncInfo(
                        on_wait=[waits[-1]], on_update=list(si.on_update)
                    )
                    changed = True
                new_insts.append(inst)
            if changed:
                bb.instructions = new_insts


_NC_CACHE = {}


def _build_nc(shifts):
    key = tuple(shifts)
    if key in _NC_CACHE:
        return _NC_CACHE[key]
    ns = len(shifts)
    nc = bass.Bass(num_devices=N_CORES)
    f16 = mybir.dt.float16
    f32 = mybir.dt.float32
    add = mybir.AluOpType.add
    mult = mybir.AluOpType.mult
    bypass = mybir.AluOpType.bypass

    uT16 = nc.dram_tensor("uT16", [HALF, L], f16, kind="ExternalInput")
    pwTo16 = nc.dram_tensor("pwTo16", [HALF, D], f16, kind="ExternalInput")
    pbh = nc.dram_tensor("pbh", [HALF, 1], f32, kind="ExternalInput")
    P32 = nc.dram_tensor("P32", [HALF, ns], f32, kind="ExternalInput")
    Q32 = nc.dram_tensor("Q32", [HALF, ns], f32, kind="ExternalInput")
    outT16 = nc.dram_tensor("outT16", [HALF, L], f16, kind="ExternalOutput")

    groups = [[2 * b, 2 * b + 1] for b in range(B)]

    with tile.TileContext(nc) as tc:
        with (
            tc.tile_pool(name="const", bufs=1) as const_pool,
            tc.tile_pool(name="scan", bufs=2) as scan_pool,
            tc.tile_pool(name="acc", bufs=2) as acc_pool,
            tc.tile_pool(name="g", bufs=4) as g_pool,
            tc.tile_pool(name="ps", bufs=4, space="PSUM") as ps_pool,
            tc.tile_pool(name="dram", bufs=1, space="DRAM") as dram_pool,
        ):
            u_t = const_pool.tile([128, KTH * L], f16)
            nc.sync.dma_start(
                out=u_t[:].rearrange("p (k l) -> p k l", k=KTH),
                in_=uT16.rearrange("(k p) l -> p k l", p=128),
            )
            pw_t = const_pool.tile([128, KTH * D], f16)
            nc.sync.dma_start(
                out=pw_t[:].rearrange("p (k o) -> p k o", k=KTH),
                in_=pwTo16.rearrange("(k p) o -> p k o", p=128),
            )
            pb_t = const_pool.tile([128, CT], f32)
            nc.sync.dma_start(
                out=pb_t[:].rearrange("p (k j) -> p k j", k=CT),
                in_=pbh.rearrange("(k p) j -> p k j", p=128),
            )
            p_t = const_pool.tile([128, CT * ns], f32)
            nc.sync.dma_start(
                out=p_t[:].rearrange("p (k j) -> p k j", k=CT),
                in_=P32.rearrange("(k p) j -> p k j", p=128),
            )
            q_t = const_pool.tile([128, CT * ns], f32)
            nc.sync.dma_start(
                out=q_t[:].rearrange("p (k j) -> p k j", k=CT),
                in_=Q32.rearrange("(k p) j -> p k j", p=128),
            )

            # partial projection over own channels, all output columns
            partial = dram_pool.tile([D, L], f16)
            projred = dram_pool.tile([HALF, L], f16)
            for oc in range(NOC):
                for lc in range(NLCH):
                    ps = ps_pool.tile([128, 512], f32)
                    for kt in range(KTH):
                        nc.tensor.matmul(
                            ps[:],
                            pw_t[:, kt * D + oc * 128 : kt * D + (oc + 1) * 128],
                            u_t[:, kt * L + lc * 512 : kt * L + lc * 512 + 512],
                            start=(kt == 0),
                            stop=(kt == KTH - 1),
                        )
                    pg = g_pool.tile([128, 512], f16, tag="pg")
                    nc.vector.tensor_copy(pg[:], ps[:])
                    nc.sync.dma_start(
                        out=partial[oc * 128 : (oc + 1) * 128, lc * 512 : (lc + 1) * 512],
                        in_=pg[:],
                    )
            nc.gpsimd.collective_compute(
                "ReduceScatter",
                add,
                replica_groups=groups,
                ins=[partial[:].opt()],
                outs=[projred[:].opt()],
            )
            proj_t = const_pool.tile([128, CT * L], f16)
            nc.sync.dma_start(
                out=proj_t[:].rearrange("p (k l) -> p k l", k=CT),
                in_=projred[:].rearrange("(k p) l -> p k l", p=128),
            )

            for ct in range(CT):
                u_ct = u_t[:, ct * L : (ct + 1) * L]
                u1 = scan_pool.tile([128, L], f32, tag="u1")
                nc.vector.tensor_tensor_scan(
                    u1[:], u_ct, u_ct, 0.0, add, bypass
                )
                u2 = scan_pool.tile([128, L], f32, tag="u2")
                nc.vector.tensor_tensor_scan(
                    u2[:], u1[:], u1[:], 0.0, add, bypass
                )
                acc = acc_pool.tile([128, L], f32)
                nc.vector.tensor_scalar_mul(
                    acc[:], u1[:], p_t[:, ct * ns : ct * ns + 1]
                )
                nc.vector.scalar_tensor_tensor(
                    acc[:], u2[:], q_t[:, ct * ns : ct * ns + 1], acc[:],
                    mult, add,
                )
                for j in range(1, ns):
                    sg = shifts[j]
                    w = L - sg
                    nc.vector.scalar_tensor_tensor(
                        acc[:, sg:], u1[:, :w],
                        p_t[:, ct * ns + j : ct * ns + j + 1],
                        acc[:, sg:], mult, add,
                    )
                    nc.vector.scalar_tensor_tensor(
                        acc[:, sg:], u2[:, :w],
                        q_t[:, ct * ns + j : ct * ns + j + 1],
                        acc[:, sg:], mult, add,
                    )
                for lc in range(NLCH):
                    lsl = slice(lc * 512, (lc + 1) * 512)
                    g = g_pool.tile([128, 512], f16, tag="g")
                    nc.vector.scalar_tensor_tensor(
                        g[:], proj_t[:, ct * L + lc * 512 : ct * L + lc * 512 + 512],
                        pb_t[:, ct : ct + 1], acc[:, lsl],
                        add, mult,
                    )
                    nc.vector.tensor_add(g[:], g[:], u_ct[:, lsl])
                    nc.sync.dma_start(
                        out=outT16[ct * 128 : (ct + 1) * 128, lsl], in_=g[:]
                    )
    _split_multi_waits(nc)
    _NC_CACHE[key] = nc
    return nc


def _conv_coeffs(z, w1, b1, w2, b2):
    """Piecewise-linear decomposition of the implicit filter (see kernel3)."""
    pe = z[0, :L].astype(np.float64)
    g = pe @ w1.T.astype(np.float64) + b1.astype(np.float64)
    s_idx = np.arange(L, dtype=np.float64)
    A = np.stack([s_idx, np.ones(L)], axis=1)
    coef, *_ = np.linalg.lstsq(A, g, rcond=None)
    if np.abs(g - A @ coef).max() > 1e-5:
        return None
    a_u, b_u = coef[0], coef[1]
    P = {0: b2.astype(np.float64).copy()}
    Q = {0: np.zeros(D, np.float64)}
    active = g > 0
    for hh in range(g.shape[1]):
        al, be = a_u[hh], b_u[hh]
        act = active[:, hh]
        if not act.any():
            continue
        w2h = w2[:, hh].astype(np.float64)
        if act.all():
            P[0] += w2h * (be - al)
            Q[0] += w2h * al
            continue
        if np.count_nonzero(act[1:] != act[:-1]) != 1:
            return None
        if act[-1] and not act[0]:
            sig = int(np.argmax(act))
            P.setdefault(sig, np.zeros(D, np.float64))
            Q.setdefault(sig, np.zeros(D, np.float64))
            P[sig] += w2h * (be + al * (sig - 1))
            Q[sig] += w2h * al
        else:
            sig = int(np.argmax(~act))
            P[0] += w2h * (be - al)
            Q[0] += w2h * al
            P.setdefault(sig, np.zeros(D, np.float64))
            Q.setdefault(sig, np.zeros(D, np.float64))
            P[sig] -= w2h * (be + al * (sig - 1))
            Q[sig] -= w2h * al
    shifts = sorted(P.keys())
    Pm = np.stack([P[s] for s in shifts]).astype(np.float32)
    Qm = np.stack([Q[s] for s in shifts]).astype(np.float32)
    return shifts, Pm, Qm


def kernel(**inputs):
    u = np.asarray(inputs["u"], dtype=np.float32)
    z = np.asarray(inputs["z"], dtype=np.float32)
    w1 = np.asarray(inputs["w1"], dtype=np.float32)
    b1 = np.asarray(inputs["b1"], dtype=np.float32)
    w2 = np.asarray(inputs["w2"], dtype=np.float32)
    b2 = np.asarray(inputs["b2"], dtype=np.float32)
    pw = np.asarray(inputs["pw"], dtype=np.float32)
    pb = np.asarray(inputs["pb"], dtype=np.float32)

    cc = _conv_coeffs(z, w1, b1, w2, b2)
    if cc is None:  # unexpected weights: exact host fallback
        pe = z[:, :L]
        h = np.maximum(np.einsum("ble,he->blh", pe, w1) + b1, 0.0)
        filt = (np.einsum("blh,dh->bld", h, w2) + b2)[0].T
        k_f = np.fft.rfft(filt, n=2 * L)
        u_t = u.transpose(0, 2, 1)
        y = np.fft.irfft(np.fft.rfft(u_t, n=2 * L) * k_f, n=2 * L)[..., :L]
        proj = np.einsum("bld,od->blo", u, pw) + pb
        return (y.transpose(0, 2, 1) * proj + u).astype(np.float32)
    shifts, Pm, Qm = cc
    ns = len(shifts)

    pwT16 = pw.T.astype(np.float16)  # (D, D), pwT[d, o] = pw[o, d]
    ut = np.ascontiguousarray(u.transpose(0, 2, 1))  # (B, D, L)

    in_maps = []
    for c in range(N_CORES):
        b, hf = c // 2, c % 2
        own = slice(hf * HALF, (hf + 1) * HALF)
        in_maps.append(
            {
                "uT16": ut[b, own].astype(np.float16),
                "pwTo16": np.ascontiguousarray(pwT16[own]),
                "pbh": pb[own].reshape(HALF, 1).astype(np.float32),
                "P32": np.ascontiguousarray(Pm[:, own].T),
                "Q32": np.ascontiguousarray(Qm[:, own].T),
            }
        )

    nc = _build_nc(shifts)
    res = run_bass_kernel_spmd(nc, in_maps, list(range(N_CORES)))

    outT = np.empty((B, D, L), dtype=np.float32)
    for c in range(N_CORES):
        b, hf = c // 2, c % 2
        outT[b, hf * HALF : (hf + 1) * HALF] = res.results[c]["outT16"]
    return outT.transpose(0, 2, 1)
